# revision 22
# baseline (speedup 1.0000x reference)
"""Trainium2 Bass kernel for GQA multi-head attention block (nn_MHA_68831145886222).

Computation (reference):
  qkv = x @ w_qkv.T ; split q[32 heads],k[8],v[8] (HD=128)
  q,k = rmsnorm(head_dim) -> rope(interleaved, theta=1e6)
  out = causal GQA attention (4 q heads per kv head)
  y   = (attn out) @ w_out.T

Sharding: tensor-parallel by kv-head group. Core g of 8 owns q heads
4g..4g+3 and kv head g (columns of the qkv projection), plus the matching
512 input rows of w_out. Each core computes a partial y [2048,4096]; the
host sums the 8 partials.

Device-side layout choices per core:
  stage 1 (qkv proj):  stationary = x^T tiles [128 d, 128 s] (bf16),
                       moving = w_qkv^T slices -> qkv in natural [s, e] psum
  postproc: rmsnorm stats via ACT Square+accum; rope via pair-swap copy +
            two muls + add (tables host-precomputed); per-head rstd apply
            (score scale 1/sqrt(HD) and ln weights folded in); PE-transpose
            q,k to [hd, s]; v stays natural [s, hd].
  attention: scoresT [k, q] = kT-tile.T @ qT (exact causal via restricted
             moving dim); exp on ACT; diagonal 128x128 blocks masked by a
             0/1 mask mul; PV and the softmax denominator (ones-matmul)
             accumulate over k-tiles in PSUM; normalize after PV.
  stage 3 (out proj): stationary = attnT [128 hd, 128 s], moving = w_out^T
             slices; accumulate over the core's 4 heads; fp32 partial out.
"""

import os
import sys
import types

import numpy as np

H = 32
G = 8
HD = 128
S = 2048
D = 4096
HG = H // G  # q heads per kv head = 4
EPS = 1e-5
THETA = 1e6
N_CORES = 8
ST = S // 128  # 16 s-tiles
DT = D // 128  # 32 d-tiles
QC = 4  # q chunks of 512
EC = 8  # e chunks of 512 in final matmul


def _ensure_ntff_hook():
    """Register the axon NTFF profile hook if the image's antenv lacks it,
    so run_bass_kernel_spmd(trace=True) can return exec_time_ns."""
    try:
        from antenv.axon_hooks import get_axon_ntff_profile_hook  # noqa: F401
        return
    except ImportError:
        pass
    try:
        import antenv
        mod = types.ModuleType("antenv.axon_hooks")
        _h = [None]
        mod.set_axon_ntff_profile_hook = lambda h: _h.__setitem__(0, h)
        mod.get_axon_ntff_profile_hook = lambda: _h[0]
        sys.modules["antenv.axon_hooks"] = mod
        antenv.axon_hooks = mod
        from trn_agent_boot.trn_boot import _ntff_profile_via_ctypes
        so = "/opt/axon/libaxon_pjrt.so"
        if os.path.exists(so):
            mod.set_axon_ntff_profile_hook(_ntff_profile_via_ctypes(so))
    except Exception:
        pass


def _build_nc():
    import concourse.bass as bass  # noqa: F401
    import concourse.tile as tile
    from concourse import bacc, mybir

    bf16 = mybir.dt.bfloat16
    f16 = mybir.dt.float16
    f32 = mybir.dt.float32
    AF = mybir.ActivationFunctionType

    nc = bacc.Bacc("TRN2", target_bir_lowering=False, debug=False,
                   num_devices=N_CORES)

    # ---- DRAM I/O ----
    xt_d = nc.dram_tensor("xt", [ST, 128, DT, 128], bf16, kind="ExternalInput").ap()
    wqkv_d = nc.dram_tensor("wqkvT", [D, 768], bf16, kind="ExternalInput").ap()
    wo_d = nc.dram_tensor("woT", [512, D], bf16, kind="ExternalInput").ap()
    ccd_d = nc.dram_tensor("ccd", [S, 128], f16, kind="ExternalInput").ap()
    ssd_d = nc.dram_tensor("ssd", [S, 128], f16, kind="ExternalInput").ap()
    mask_d = nc.dram_tensor("dmask", [128, 128], bf16, kind="ExternalInput").ap()
    ident_d = nc.dram_tensor("ident", [128, 128], bf16, kind="ExternalInput").ap()
    out_d = nc.dram_tensor("out", [S, D], f32, kind="ExternalOutput").ap()

    from contextlib import ExitStack
    with tile.TileContext(nc) as tc, ExitStack() as ctx:
        const = ctx.enter_context(tc.tile_pool(name="const", bufs=1))
        persist = ctx.enter_context(tc.tile_pool(name="persist", bufs=1))
        xpool = ctx.enter_context(tc.tile_pool(name="xpool", bufs=3))
        scratch = ctx.enter_context(tc.tile_pool(name="scratch", bufs=2))
        small = ctx.enter_context(tc.tile_pool(name="small", bufs=2))
        epool = ctx.enter_context(tc.tile_pool(name="epool", bufs=7))
        opool = ctx.enter_context(tc.tile_pool(name="opool", bufs=2))
        psum = ctx.enter_context(tc.tile_pool(name="psum", bufs=4, space="PSUM"))

        # ---- critical path to first matmul: first x tile + first wq chunk ----
        wq_sb = persist.tile([128, DT, 768], bf16, tag="bigw")
        wq_r = wqkv_d.rearrange("(t p) e -> p t e", p=128)
        xs_pre = []
        for st0 in range(3):
            xs_p = xpool.tile([128, DT, 128], bf16, name="xs")
            if st0 == 0:
                for c0, c1 in [(0, 4), (4, 8), (8, 16), (16, 24), (24, 32)]:
                    nc.sync.dma_start(out=xs_p[:, c0:c1, :],
                                      in_=xt_d[st0, :, c0:c1, :])
            else:
                nc.sync.dma_start(out=xs_p, in_=xt_d[st0])
            xs_pre.append(xs_p)
        nc.sync.dma_start(out=wq_sb[:, 0:1, :], in_=wq_r[:, 0:1, :])
        nc.sync.dma_start(out=wq_sb[:, 1:2, :], in_=wq_r[:, 1:2, :])
        nc.sync.dma_start(out=wq_sb[:, 2:4, :], in_=wq_r[:, 2:4, :])
        for dtc in range(4, DT, 4):
            nc.sync.dma_start(out=wq_sb[:, dtc:dtc + 4, :],
                              in_=wq_r[:, dtc:dtc + 4, :])

        # ---- constants / persistent tensors ----
        ccd_sb = const.tile([128, ST, 128], f16)
        nc.sync.dma_start(out=ccd_sb, in_=ccd_d.rearrange("(t p) h -> p t h", p=128))
        ssd_sb = const.tile([128, ST, 128], f16)
        nc.sync.dma_start(out=ssd_sb, in_=ssd_d.rearrange("(t p) h -> p t h", p=128))
        mask_sb = const.tile([128, 128], bf16)
        nc.sync.dma_start(out=mask_sb, in_=mask_d)
        ident_sb = const.tile([128, 128], bf16)
        nc.sync.dma_start(out=ident_sb, in_=ident_d)
        onesm_sb = const.tile([128, 128], bf16)
        nc.vector.memset(onesm_sb, 1.0)
        bias_q = const.tile([128, 1], f32)
        nc.vector.memset(bias_q, float(HD * EPS))
        bias_k = const.tile([128, 1], f32)
        nc.vector.memset(bias_k, float(EPS))

        qT_sb = persist.tile([128, HG, S], bf16)   # [hd, head, s]
        kT_sb = persist.tile([128, S], bf16)       # [hd, s]
        v_sb = persist.tile([128, ST, 128], bf16)  # [s_local, s_tile, hd]
        oT_sb = persist.tile([128, HG, S], bf16)   # attn outT [hd, head, s]

        # ================= stage 1: qkv projection + postproc ==============
        for st in range(ST):
            if st < 3:
                xs = xs_pre[st]
            else:
                xs = xpool.tile([128, DT, 128], bf16, name="xs")
                nc.sync.dma_start(out=xs, in_=xt_d[st])

            q_ps = psum.tile([128, 512], f32, tag="pa", bufs=2)
            kv_ps = psum.tile([128, 512], f32, tag="pd", bufs=2)
            for dt_i in range(DT):
                nc.tensor.matmul(q_ps, xs[:, dt_i, :], wq_sb[:, dt_i, 0:512],
                                 start=(dt_i == 0), stop=(dt_i == DT - 1))
                nc.tensor.matmul(kv_ps[:, 0:256], xs[:, dt_i, :], wq_sb[:, dt_i, 512:768],
                                 start=(dt_i == 0), stop=(dt_i == DT - 1))

            # v: straight cast copy to [s, hd]
            nc.vector.tensor_copy(out=v_sb[:, st, :], in_=kv_ps[:, 128:256])

            # sum of squares per head (ACT Square with free-dim accumulate)
            ssq = small.tile([128, 5], f32)
            sqs = scratch.tile([128, 512], f32)
            for hh in range(HG):
                nc.scalar.activation(out=sqs[:, hh * 128:(hh + 1) * 128],
                                     in_=q_ps[:, hh * 128:(hh + 1) * 128],
                                     func=AF.Square,
                                     accum_out=ssq[:, hh:hh + 1])
            sqk = small.tile([128, 128], f32)
            nc.scalar.activation(out=sqk, in_=kv_ps[:, 0:128], func=AF.Square,
                                 accum_out=ssq[:, 4:5])
            # rstd: q gets the 1/sqrt(HD) score scale folded in
            rstd = small.tile([128, 5], f32)
            nc.scalar.activation(out=rstd[:, 0:4], in_=ssq[:, 0:4],
                                 func=AF.Sqrt, bias=bias_q, scale=1.0)
            nc.scalar.activation(out=rstd[:, 4:5], in_=ssq[:, 4:5],
                                 func=AF.Sqrt, bias=bias_k, scale=1.0 / HD)
            nc.vector.reciprocal(out=rstd, in_=rstd)

            # rope q (4 heads batched; tables broadcast over head dim)
            q4 = q_ps.rearrange("p (h r two) -> p h r two", h=HG, two=2)
            rot_q = scratch.tile([128, HG, 64, 2], f32)
            nc.vector.tensor_copy(out=rot_q, in_=q4[:, :, :, ::-1])
            cc_b = ccd_sb[:, st, :].unsqueeze(1).broadcast_to((128, HG, 128))
            ss_b = ssd_sb[:, st, :].unsqueeze(1).broadcast_to((128, HG, 128))
            qcc = scratch.tile([128, HG, 128], f32)
            nc.vector.tensor_mul(qcc, q_ps.rearrange("p (h e) -> p h e", h=HG), cc_b)
            qss = scratch.tile([128, HG, 128], f32)
            nc.vector.tensor_mul(qss, rot_q.rearrange("p h r two -> p h (r two)"), ss_b)
            qrope = scratch.tile([128, HG, 128], f32)
            nc.vector.tensor_add(qrope, qcc, qss)
            qfin = scratch.tile([128, HG, 128], bf16)
            for hh in range(HG):
                nc.vector.tensor_scalar_mul(qfin[:, hh, :], qrope[:, hh, :],
                                            rstd[:, hh:hh + 1])

            # rope k
            k1 = kv_ps[:, 0:128].rearrange("p (r two) -> p r two", two=2)
            rot_k = small.tile([128, 64, 2], f32)
            nc.vector.tensor_copy(out=rot_k, in_=k1[:, :, ::-1])
            kcc = small.tile([128, 128], f32)
            nc.vector.tensor_mul(kcc, kv_ps[:, 0:128], ccd_sb[:, st, :])
            kss = small.tile([128, 128], f32)
            nc.vector.tensor_mul(kss, rot_k.rearrange("p r two -> p (r two)"),
                                 ssd_sb[:, st, :])
            krope = small.tile([128, 128], f32)
            nc.vector.tensor_add(krope, kcc, kss)
            kfin = small.tile([128, 128], bf16)
            nc.vector.tensor_scalar_mul(kfin, krope, rstd[:, 4:5])

            # transpose q heads and k into [hd, s] layout
            for hh in range(HG):
                tq_ps = psum.tile([128, 128], bf16, tag="pb", bufs=1)
                nc.tensor.transpose(tq_ps, qfin[:, hh, :], ident_sb)
                nc.scalar.copy(out=qT_sb[:, hh, st * 128:(st + 1) * 128], in_=tq_ps)
            tk_ps = psum.tile([128, 128], bf16, tag="pb", bufs=1)
            nc.tensor.transpose(tk_ps, kfin, ident_sb)
            nc.scalar.copy(out=kT_sb[:, st * 128:(st + 1) * 128], in_=tk_ps)

        # ================= stage 3 weights (loaded during attention) =======
        wo_sb = persist.tile([128, HG, D], bf16, tag="bigw")
        nc.sync.dma_start(out=wo_sb, in_=wo_d.rearrange("(h p) e -> p h e", p=128))

        def emit_wout(st):
            out_sb = opool.tile([128, D], f32, name="out_sb")
            for half in range(2):
                o_ps = [psum.tile([128, 512], f32,
                                  tag=("pa" if i % 2 == 0 else "pc"),
                                  bufs=(2 if i % 2 == 0 else 3),
                                  name=f"o_ps_{st}_{half}_{i}")
                        for i in range(4)]
                for h in range(HG):
                    for i in range(4):
                        ec = half * 4 + i
                        nc.tensor.matmul(o_ps[i],
                                         oT_sb[:, h, st * 128:(st + 1) * 128],
                                         wo_sb[:, h, ec * 512:(ec + 1) * 512],
                                         start=(h == 0), stop=(h == HG - 1))
                for i in range(4):
                    ec = half * 4 + i
                    nc.vector.tensor_copy(
                        out=out_sb[:, ec * 512:(ec + 1) * 512], in_=o_ps[i])
                    if st == ST - 1 and i % 2 == 1:
                        q0 = half * 2048 + (i - 1) * 512
                        nc.sync.dma_start(
                            out=out_d[st * 128:(st + 1) * 128, q0:q0 + 1024],
                            in_=out_sb[:, q0:q0 + 1024])
                if st != ST - 1:
                    nc.sync.dma_start(
                        out=out_d[st * 128:(st + 1) * 128,
                                  half * 2048:(half + 1) * 2048],
                        in_=out_sb[:, half * 2048:(half + 1) * 2048])

        # ================= stage 2: attention (+ interleaved out proj) =====
        for qc in range(QC):
            for hp in range(HG // 2):
                hh0 = 2 * hp
                pv0 = psum.tile([128, 512], f32, tag="pa", bufs=2, name=f"pv0_{qc}_{hp}")
                pv1 = psum.tile([128, 512], f32, tag="pa", bufs=2, name=f"pv1_{qc}_{hp}")
                den0 = psum.tile([128, 512], f32, tag="pd", bufs=2, name=f"den0_{qc}_{hp}")
                den1 = psum.tile([128, 512], f32, tag="pd", bufs=2, name=f"den1_{qc}_{hp}")
                pvs, dens = [pv0, pv1], [den0, den1]
                n_kt = 4 * qc + 4
                for kt in range(n_kt):
                    j = kt - 4 * qc
                    off = 0 if j < 0 else 128 * j
                    exs = []
                    for hi in range(2):
                        h = hh0 + hi
                        sc_ps = psum.tile([128, 512], f32, tag="pc", bufs=3,
                                          name=f"sc_{qc}_{hp}_{kt}_{hi}")
                        nc.tensor.matmul(
                            sc_ps[:, off:512],
                            kT_sb[:, kt * 128:(kt + 1) * 128],
                            qT_sb[:, h, qc * 512 + off:(qc + 1) * 512],
                            start=True, stop=True)
                        ex = epool.tile([128, 512], bf16, name=f"ex_{hi}")
                        nc.scalar.activation(out=ex[:, off:512],
                                             in_=sc_ps[:, off:512], func=AF.Exp)
                        if j >= 0:
                            nc.vector.tensor_mul(ex[:, off:off + 128],
                                                 ex[:, off:off + 128], mask_sb)
                        exs.append(ex)
                    for hi in range(2):
                        nc.tensor.matmul(pvs[hi][:, off:512], v_sb[:, kt, :],
                                         exs[hi][:, off:512],
                                         start=(kt == 0), stop=(kt == n_kt - 1))
                    for hi in range(2):
                        nc.tensor.matmul(dens[hi][:, off:512], onesm_sb,
                                         exs[hi][:, off:512],
                                         start=(kt == 0), stop=(kt == n_kt - 1))
                for hi in range(2):
                    h = hh0 + hi
                    rden = scratch.tile([128, 512], f32, tag="rden")
                    nc.vector.reciprocal_approx_fast(out=rden, in_=dens[hi])
                    nc.vector.tensor_mul(oT_sb[:, h, qc * 512:(qc + 1) * 512],
                                         pvs[hi], rden)
            for st in range(4 * qc, 4 * qc + 4):
                emit_wout(st)


    nc.compile()
    return nc


def _host_prep(x, w_qkv, w_out, q_ln_w, k_ln_w):
    """Build per-core input maps (host-side shard + transform)."""
    bf = np.dtype("bfloat16") if hasattr(np, "bfloat16") else None
    import ml_dtypes
    bf16 = ml_dtypes.bfloat16

    x2 = np.asarray(x, np.float32).reshape(S, D)
    # x tiles [st, d_local, d_tile, s_local] so each s-tile DMA is contiguous
    xt = np.ascontiguousarray(
        x2.reshape(ST, 128, DT, 128).transpose(0, 3, 2, 1)).astype(bf16)

    # rope tables (duplicated cos / sign-baked sin, interleaved layout)
    freqs = 1.0 / (THETA ** (np.arange(0, HD, 2, dtype=np.float64) / HD))
    ang = np.arange(S, dtype=np.float64)[:, None] * freqs[None, :]
    cos = np.cos(ang).astype(np.float32)
    sin = np.sin(ang).astype(np.float32)
    ccd = np.repeat(cos, 2, axis=1).astype(np.float16)    # [S, 128]
    ssd = np.stack([-sin, sin], axis=-1).reshape(S, HD).astype(np.float16)

    kq = np.arange(128)
    dmask = (kq[:, None] <= kq[None, :]).astype(bf16)     # [k, q]
    ident = np.eye(128, dtype=bf16)

    wq = np.asarray(w_qkv, np.float32)
    wo = np.asarray(w_out, np.float32)
    qw = np.asarray(q_ln_w, np.float32)
    kw = np.asarray(k_ln_w, np.float32)

    in_maps = []
    for g in range(N_CORES):
        wq_g = wq[512 * g:512 * (g + 1), :].reshape(HG, HD, D) * qw[None, :, None]
        wk_g = wq[D + 128 * g:D + 128 * (g + 1), :] * kw[:, None]
        wv_g = wq[D + G * HD + 128 * g:D + G * HD + 128 * (g + 1), :]
        wqkv_g = np.concatenate([wq_g.reshape(512, D), wk_g, wv_g], axis=0)
        wqkvT_g = np.ascontiguousarray(wqkv_g.T).astype(bf16)     # [D, 768]
        woT_g = np.ascontiguousarray(wo[:, 512 * g:512 * (g + 1)].T).astype(bf16)
        in_maps.append({
            "xt": xt,
            "wqkvT": wqkvT_g,
            "woT": woT_g,
            "ccd": ccd,
            "ssd": ssd,
            "dmask": dmask,
            "ident": ident,
        })
    return in_maps


_CACHE = {}


def _get_compiled():
    if "nc" not in _CACHE:
        _ensure_ntff_hook()
        _CACHE["nc"] = _build_nc()
    return _CACHE["nc"]


def run_sharded(x, w_qkv, w_out, q_ln_w, k_ln_w, trace=False):
    from concourse.bass_utils import run_bass_kernel_spmd
    nc = _get_compiled()
    in_maps = _host_prep(x, w_qkv, w_out, q_ln_w, k_ln_w)
    res = run_bass_kernel_spmd(nc, in_maps, core_ids=list(range(N_CORES)),
                               trace=trace)
    acc = np.zeros((S, D), np.float32)
    for i in range(N_CORES):
        acc += np.asarray(res.results[i]["out"], np.float32)
    return acc.reshape(1, S, D), res


def kernel(x, w_qkv, w_out, q_ln_w, k_ln_w):
    out, _ = run_sharded(x, w_qkv, w_out, q_ln_w, k_ln_w, trace=False)
    return out


# revision 23
# speedup vs baseline: 1.0039x; 1.0039x over previous
"""Trainium2 Bass kernel for GQA multi-head attention block (nn_MHA_68831145886222).

Computation (reference):
  qkv = x @ w_qkv.T ; split q[32 heads],k[8],v[8] (HD=128)
  q,k = rmsnorm(head_dim) -> rope(interleaved, theta=1e6)
  out = causal GQA attention (4 q heads per kv head)
  y   = (attn out) @ w_out.T

Sharding: tensor-parallel by kv-head group. Core g of 8 owns q heads
4g..4g+3 and kv head g (columns of the qkv projection), plus the matching
512 input rows of w_out. Each core computes a partial y [2048,4096]; the
host sums the 8 partials.

Device-side layout choices per core:
  stage 1 (qkv proj):  stationary = x^T tiles [128 d, 128 s] (bf16),
                       moving = w_qkv^T slices -> qkv in natural [s, e] psum
  postproc: rmsnorm stats via ACT Square+accum; rope via pair-swap copy +
            two muls + add (tables host-precomputed); per-head rstd apply
            (score scale 1/sqrt(HD) and ln weights folded in); PE-transpose
            q,k to [hd, s]; v stays natural [s, hd].
  attention: scoresT [k, q] = kT-tile.T @ qT (exact causal via restricted
             moving dim); exp on ACT; diagonal 128x128 blocks masked by a
             0/1 mask mul; PV and the softmax denominator (ones-matmul)
             accumulate over k-tiles in PSUM; normalize after PV.
  stage 3 (out proj): stationary = attnT [128 hd, 128 s], moving = w_out^T
             slices; accumulate over the core's 4 heads; fp32 partial out.
"""

import os
import sys
import types

import numpy as np

H = 32
G = 8
HD = 128
S = 2048
D = 4096
HG = H // G  # q heads per kv head = 4
EPS = 1e-5
THETA = 1e6
N_CORES = 8
ST = S // 128  # 16 s-tiles
DT = D // 128  # 32 d-tiles
QC = 4  # q chunks of 512
EC = 8  # e chunks of 512 in final matmul


def _ensure_ntff_hook():
    """Register the axon NTFF profile hook if the image's antenv lacks it,
    so run_bass_kernel_spmd(trace=True) can return exec_time_ns."""
    try:
        from antenv.axon_hooks import get_axon_ntff_profile_hook  # noqa: F401
        return
    except ImportError:
        pass
    try:
        import antenv
        mod = types.ModuleType("antenv.axon_hooks")
        _h = [None]
        mod.set_axon_ntff_profile_hook = lambda h: _h.__setitem__(0, h)
        mod.get_axon_ntff_profile_hook = lambda: _h[0]
        sys.modules["antenv.axon_hooks"] = mod
        antenv.axon_hooks = mod
        from trn_agent_boot.trn_boot import _ntff_profile_via_ctypes
        so = "/opt/axon/libaxon_pjrt.so"
        if os.path.exists(so):
            mod.set_axon_ntff_profile_hook(_ntff_profile_via_ctypes(so))
    except Exception:
        pass


def _build_nc():
    import concourse.bass as bass  # noqa: F401
    import concourse.tile as tile
    from concourse import bacc, mybir

    bf16 = mybir.dt.bfloat16
    f16 = mybir.dt.float16
    f32 = mybir.dt.float32
    AF = mybir.ActivationFunctionType

    nc = bacc.Bacc("TRN2", target_bir_lowering=False, debug=False,
                   num_devices=N_CORES)

    # ---- DRAM I/O ----
    xt_d = nc.dram_tensor("xt", [ST, 128, DT, 128], bf16, kind="ExternalInput").ap()
    wqkv_d = nc.dram_tensor("wqkvT", [D, 768], bf16, kind="ExternalInput").ap()
    wo_d = nc.dram_tensor("woT", [512, D], bf16, kind="ExternalInput").ap()
    ccd_d = nc.dram_tensor("ccd", [S, 128], f16, kind="ExternalInput").ap()
    ssd_d = nc.dram_tensor("ssd", [S, 128], f16, kind="ExternalInput").ap()
    mask_d = nc.dram_tensor("dmask", [128, 128], bf16, kind="ExternalInput").ap()
    ident_d = nc.dram_tensor("ident", [128, 128], bf16, kind="ExternalInput").ap()
    out_d = nc.dram_tensor("out", [S, D], f32, kind="ExternalOutput").ap()

    from contextlib import ExitStack
    with tile.TileContext(nc) as tc, ExitStack() as ctx:
        const = ctx.enter_context(tc.tile_pool(name="const", bufs=1))
        persist = ctx.enter_context(tc.tile_pool(name="persist", bufs=1))
        xpool = ctx.enter_context(tc.tile_pool(name="xpool", bufs=3))
        scratch = ctx.enter_context(tc.tile_pool(name="scratch", bufs=2))
        small = ctx.enter_context(tc.tile_pool(name="small", bufs=2))
        epool = ctx.enter_context(tc.tile_pool(name="epool", bufs=7))
        opool = ctx.enter_context(tc.tile_pool(name="opool", bufs=2))
        psum = ctx.enter_context(tc.tile_pool(name="psum", bufs=4, space="PSUM"))

        # ---- critical path to first matmul: first x tile + first wq chunk ----
        wq_sb = persist.tile([128, DT, 768], bf16, tag="bigw")
        wq_r = wqkv_d.rearrange("(t p) e -> p t e", p=128)
        xs_pre = []
        for st0 in range(3):
            xs_p = xpool.tile([128, DT, 128], bf16, name="xs")
            if st0 == 0:
                for c0, c1 in [(0, 4), (4, 8), (8, 16), (16, 24), (24, 32)]:
                    nc.sync.dma_start(out=xs_p[:, c0:c1, :],
                                      in_=xt_d[st0, :, c0:c1, :])
            else:
                nc.sync.dma_start(out=xs_p, in_=xt_d[st0])
            xs_pre.append(xs_p)
        nc.sync.dma_start(out=wq_sb[:, 0:1, :], in_=wq_r[:, 0:1, :])
        nc.sync.dma_start(out=wq_sb[:, 1:2, :], in_=wq_r[:, 1:2, :])
        nc.sync.dma_start(out=wq_sb[:, 2:4, :], in_=wq_r[:, 2:4, :])
        for dtc in range(4, DT, 4):
            nc.sync.dma_start(out=wq_sb[:, dtc:dtc + 4, :],
                              in_=wq_r[:, dtc:dtc + 4, :])

        # ---- constants / persistent tensors ----
        ccd_sb = const.tile([128, ST, 128], f16)
        nc.sync.dma_start(out=ccd_sb, in_=ccd_d.rearrange("(t p) h -> p t h", p=128))
        ssd_sb = const.tile([128, ST, 128], f16)
        nc.sync.dma_start(out=ssd_sb, in_=ssd_d.rearrange("(t p) h -> p t h", p=128))
        mask_sb = const.tile([128, 128], bf16)
        nc.sync.dma_start(out=mask_sb, in_=mask_d)
        ident_sb = const.tile([128, 128], bf16)
        nc.sync.dma_start(out=ident_sb, in_=ident_d)
        onesm_sb = const.tile([128, 128], bf16)
        nc.vector.memset(onesm_sb, 1.0)
        bias_q = const.tile([128, 1], f32)
        nc.vector.memset(bias_q, float(HD * EPS))
        bias_k = const.tile([128, 1], f32)
        nc.vector.memset(bias_k, float(EPS))

        qT_sb = persist.tile([128, HG, S], bf16)   # [hd, head, s]
        kT_sb = persist.tile([128, S], bf16)       # [hd, s]
        v_sb = persist.tile([128, ST, 128], bf16)  # [s_local, s_tile, hd]
        oT_sb = persist.tile([128, HG, S], bf16)   # attn outT [hd, head, s]

        # ================= stage 1: qkv projection + postproc ==============
        for st in range(ST):
            if st < 3:
                xs = xs_pre[st]
            else:
                xs = xpool.tile([128, DT, 128], bf16, name="xs")
                nc.sync.dma_start(out=xs, in_=xt_d[st])

            q_ps = psum.tile([128, 512], f32, tag="pa", bufs=3)
            kv_ps = psum.tile([128, 512], f32, tag="pd", bufs=2)
            for dt_i in range(DT):
                nc.tensor.matmul(q_ps, xs[:, dt_i, :], wq_sb[:, dt_i, 0:512],
                                 start=(dt_i == 0), stop=(dt_i == DT - 1))
                nc.tensor.matmul(kv_ps[:, 0:256], xs[:, dt_i, :], wq_sb[:, dt_i, 512:768],
                                 start=(dt_i == 0), stop=(dt_i == DT - 1))

            # v: straight cast copy to [s, hd]
            nc.vector.tensor_copy(out=v_sb[:, st, :], in_=kv_ps[:, 128:256])

            # sum of squares per head (ACT Square with free-dim accumulate)
            ssq = small.tile([128, 5], f32)
            sqs = scratch.tile([128, 512], f32)
            for hh in range(HG):
                nc.scalar.activation(out=sqs[:, hh * 128:(hh + 1) * 128],
                                     in_=q_ps[:, hh * 128:(hh + 1) * 128],
                                     func=AF.Square,
                                     accum_out=ssq[:, hh:hh + 1])
            sqk = small.tile([128, 128], f32)
            nc.scalar.activation(out=sqk, in_=kv_ps[:, 0:128], func=AF.Square,
                                 accum_out=ssq[:, 4:5])
            # rstd: q gets the 1/sqrt(HD) score scale folded in
            rstd = small.tile([128, 5], f32)
            nc.scalar.activation(out=rstd[:, 0:4], in_=ssq[:, 0:4],
                                 func=AF.Sqrt, bias=bias_q, scale=1.0)
            nc.scalar.activation(out=rstd[:, 4:5], in_=ssq[:, 4:5],
                                 func=AF.Sqrt, bias=bias_k, scale=1.0 / HD)
            nc.vector.reciprocal(out=rstd, in_=rstd)

            # rope q (4 heads batched; tables broadcast over head dim)
            q4 = q_ps.rearrange("p (h r two) -> p h r two", h=HG, two=2)
            rot_q = scratch.tile([128, HG, 64, 2], f32)
            nc.vector.tensor_copy(out=rot_q, in_=q4[:, :, :, ::-1])
            cc_b = ccd_sb[:, st, :].unsqueeze(1).broadcast_to((128, HG, 128))
            ss_b = ssd_sb[:, st, :].unsqueeze(1).broadcast_to((128, HG, 128))
            qcc = scratch.tile([128, HG, 128], f32)
            nc.vector.tensor_mul(qcc, q_ps.rearrange("p (h e) -> p h e", h=HG), cc_b)
            qss = scratch.tile([128, HG, 128], f32)
            nc.vector.tensor_mul(qss, rot_q.rearrange("p h r two -> p h (r two)"), ss_b)
            qrope = scratch.tile([128, HG, 128], f32)
            nc.vector.tensor_add(qrope, qcc, qss)
            qfin = scratch.tile([128, HG, 128], bf16)
            for hh in range(HG):
                nc.vector.tensor_scalar_mul(qfin[:, hh, :], qrope[:, hh, :],
                                            rstd[:, hh:hh + 1])

            # rope k
            k1 = kv_ps[:, 0:128].rearrange("p (r two) -> p r two", two=2)
            rot_k = small.tile([128, 64, 2], f32)
            nc.vector.tensor_copy(out=rot_k, in_=k1[:, :, ::-1])
            kcc = small.tile([128, 128], f32)
            nc.vector.tensor_mul(kcc, kv_ps[:, 0:128], ccd_sb[:, st, :])
            kss = small.tile([128, 128], f32)
            nc.vector.tensor_mul(kss, rot_k.rearrange("p r two -> p (r two)"),
                                 ssd_sb[:, st, :])
            krope = small.tile([128, 128], f32)
            nc.vector.tensor_add(krope, kcc, kss)
            kfin = small.tile([128, 128], bf16)
            nc.vector.tensor_scalar_mul(kfin, krope, rstd[:, 4:5])

            # transpose q heads and k into [hd, s] layout
            for hh in range(HG):
                tq_ps = psum.tile([128, 128], bf16, tag="pd", bufs=2)
                nc.tensor.transpose(tq_ps, qfin[:, hh, :], ident_sb)
                nc.scalar.copy(out=qT_sb[:, hh, st * 128:(st + 1) * 128], in_=tq_ps)
            tk_ps = psum.tile([128, 128], bf16, tag="pd", bufs=2)
            nc.tensor.transpose(tk_ps, kfin, ident_sb)
            nc.scalar.copy(out=kT_sb[:, st * 128:(st + 1) * 128], in_=tk_ps)

        # ================= stage 3 weights (loaded during attention) =======
        wo_sb = persist.tile([128, HG, D], bf16, tag="bigw")
        nc.sync.dma_start(out=wo_sb, in_=wo_d.rearrange("(h p) e -> p h e", p=128))

        def emit_wout(st):
            out_sb = opool.tile([128, D], f32, name="out_sb")
            for half in range(2):
                o_ps = [psum.tile([128, 512], f32,
                                  tag=("pa" if i % 2 == 0 else "pc"),
                                  bufs=(3 if i % 2 == 0 else 3),
                                  name=f"o_ps_{st}_{half}_{i}")
                        for i in range(4)]
                for h in range(HG):
                    for i in range(4):
                        ec = half * 4 + i
                        nc.tensor.matmul(o_ps[i],
                                         oT_sb[:, h, st * 128:(st + 1) * 128],
                                         wo_sb[:, h, ec * 512:(ec + 1) * 512],
                                         start=(h == 0), stop=(h == HG - 1))
                for i in range(4):
                    ec = half * 4 + i
                    nc.vector.tensor_copy(
                        out=out_sb[:, ec * 512:(ec + 1) * 512], in_=o_ps[i])
                    if st == ST - 1 and i % 2 == 1:
                        q0 = half * 2048 + (i - 1) * 512
                        nc.sync.dma_start(
                            out=out_d[st * 128:(st + 1) * 128, q0:q0 + 1024],
                            in_=out_sb[:, q0:q0 + 1024])
                if st != ST - 1:
                    nc.sync.dma_start(
                        out=out_d[st * 128:(st + 1) * 128,
                                  half * 2048:(half + 1) * 2048],
                        in_=out_sb[:, half * 2048:(half + 1) * 2048])

        # ================= stage 2: attention (+ interleaved out proj) =====
        for qc in range(QC):
            for hp in range(HG // 2):
                hh0 = 2 * hp
                pv0 = psum.tile([128, 512], f32, tag="pa", bufs=3, name=f"pv0_{qc}_{hp}")
                pv1 = psum.tile([128, 512], f32, tag="pa", bufs=3, name=f"pv1_{qc}_{hp}")
                den0 = psum.tile([128, 512], f32, tag="pd", bufs=2, name=f"den0_{qc}_{hp}")
                den1 = psum.tile([128, 512], f32, tag="pd", bufs=2, name=f"den1_{qc}_{hp}")
                pvs, dens = [pv0, pv1], [den0, den1]
                n_kt = 4 * qc + 4
                for kt in range(n_kt):
                    j = kt - 4 * qc
                    off = 0 if j < 0 else 128 * j
                    exs = []
                    for hi in range(2):
                        h = hh0 + hi
                        sc_ps = psum.tile([128, 512], f32, tag="pc", bufs=3,
                                          name=f"sc_{qc}_{hp}_{kt}_{hi}")
                        nc.tensor.matmul(
                            sc_ps[:, off:512],
                            kT_sb[:, kt * 128:(kt + 1) * 128],
                            qT_sb[:, h, qc * 512 + off:(qc + 1) * 512],
                            start=True, stop=True)
                        ex = epool.tile([128, 512], bf16, name=f"ex_{hi}")
                        nc.scalar.activation(out=ex[:, off:512],
                                             in_=sc_ps[:, off:512], func=AF.Exp)
                        if j >= 0:
                            nc.vector.tensor_mul(ex[:, off:off + 128],
                                                 ex[:, off:off + 128], mask_sb)
                        exs.append(ex)
                    for hi in range(2):
                        nc.tensor.matmul(pvs[hi][:, off:512], v_sb[:, kt, :],
                                         exs[hi][:, off:512],
                                         start=(kt == 0), stop=(kt == n_kt - 1))
                    for hi in range(2):
                        nc.tensor.matmul(dens[hi][:, off:512], onesm_sb,
                                         exs[hi][:, off:512],
                                         start=(kt == 0), stop=(kt == n_kt - 1))
                for hi in range(2):
                    h = hh0 + hi
                    rden = scratch.tile([128, 512], f32, tag="rden")
                    nc.vector.reciprocal_approx_fast(out=rden, in_=dens[hi])
                    nc.vector.tensor_mul(oT_sb[:, h, qc * 512:(qc + 1) * 512],
                                         pvs[hi], rden)
            for st in range(4 * qc, 4 * qc + 4):
                emit_wout(st)


    nc.compile()
    return nc


def _host_prep(x, w_qkv, w_out, q_ln_w, k_ln_w):
    """Build per-core input maps (host-side shard + transform)."""
    bf = np.dtype("bfloat16") if hasattr(np, "bfloat16") else None
    import ml_dtypes
    bf16 = ml_dtypes.bfloat16

    x2 = np.asarray(x, np.float32).reshape(S, D)
    # x tiles [st, d_local, d_tile, s_local] so each s-tile DMA is contiguous
    xt = np.ascontiguousarray(
        x2.reshape(ST, 128, DT, 128).transpose(0, 3, 2, 1)).astype(bf16)

    # rope tables (duplicated cos / sign-baked sin, interleaved layout)
    freqs = 1.0 / (THETA ** (np.arange(0, HD, 2, dtype=np.float64) / HD))
    ang = np.arange(S, dtype=np.float64)[:, None] * freqs[None, :]
    cos = np.cos(ang).astype(np.float32)
    sin = np.sin(ang).astype(np.float32)
    ccd = np.repeat(cos, 2, axis=1).astype(np.float16)    # [S, 128]
    ssd = np.stack([-sin, sin], axis=-1).reshape(S, HD).astype(np.float16)

    kq = np.arange(128)
    dmask = (kq[:, None] <= kq[None, :]).astype(bf16)     # [k, q]
    ident = np.eye(128, dtype=bf16)

    wq = np.asarray(w_qkv, np.float32)
    wo = np.asarray(w_out, np.float32)
    qw = np.asarray(q_ln_w, np.float32)
    kw = np.asarray(k_ln_w, np.float32)

    in_maps = []
    for g in range(N_CORES):
        wq_g = wq[512 * g:512 * (g + 1), :].reshape(HG, HD, D) * qw[None, :, None]
        wk_g = wq[D + 128 * g:D + 128 * (g + 1), :] * kw[:, None]
        wv_g = wq[D + G * HD + 128 * g:D + G * HD + 128 * (g + 1), :]
        wqkv_g = np.concatenate([wq_g.reshape(512, D), wk_g, wv_g], axis=0)
        wqkvT_g = np.ascontiguousarray(wqkv_g.T).astype(bf16)     # [D, 768]
        woT_g = np.ascontiguousarray(wo[:, 512 * g:512 * (g + 1)].T).astype(bf16)
        in_maps.append({
            "xt": xt,
            "wqkvT": wqkvT_g,
            "woT": woT_g,
            "ccd": ccd,
            "ssd": ssd,
            "dmask": dmask,
            "ident": ident,
        })
    return in_maps


_CACHE = {}


def _get_compiled():
    if "nc" not in _CACHE:
        _ensure_ntff_hook()
        _CACHE["nc"] = _build_nc()
    return _CACHE["nc"]


def run_sharded(x, w_qkv, w_out, q_ln_w, k_ln_w, trace=False):
    from concourse.bass_utils import run_bass_kernel_spmd
    nc = _get_compiled()
    in_maps = _host_prep(x, w_qkv, w_out, q_ln_w, k_ln_w)
    res = run_bass_kernel_spmd(nc, in_maps, core_ids=list(range(N_CORES)),
                               trace=trace)
    acc = np.zeros((S, D), np.float32)
    for i in range(N_CORES):
        acc += np.asarray(res.results[i]["out"], np.float32)
    return acc.reshape(1, S, D), res


def kernel(x, w_qkv, w_out, q_ln_w, k_ln_w):
    out, _ = run_sharded(x, w_qkv, w_out, q_ln_w, k_ln_w, trace=False)
    return out
